# revision 30
# baseline (speedup 1.0000x reference)
"""Multi-head causal attention (B=2, S=2048, D=1024, H=16, dh=64) on 8
Trainium2 NeuronCores.

Sharding: core i handles batch b = i//4 and head group g = i%4 (4 heads
each).  Per core everything is computed in a transposed layout:

  QT = Wq_g^T @ x_b^T          [256(hk), 2048(S)]   (bf16)
  KT = Wk_g^T @ x_b^T          [256(hk), 2048(S)]   (bf16)
  V  = x_b @ Wv_g              [2048(S), 4, 65]     (bf16; col 64 = ones)
  per chunk c (512 queries), head-pair hp, key block j (128 keys):
     scT[par] = KT_h[:,j]^T(lhsT) x QT_h[:,c]   -> PSUM [128, 2, 512]
     expT     = exp(scT/8) (* causal mask when j >= 4c)        (bf16)
     zT_h    += V_aug[j]^T(lhsT) x expT[par]    -> PSUM [65, 512]
     ztn      = zT[0:64] * broadcast(1/s)       [256(hk), 2048] (bf16)
  outT = Wo_g^T(lhsT) x ztn                     [1024(d), 2048] (bf16)

Scheduling: the TRN2 PE p-state only reaches full clock under sustained
back-to-back execution, and the sc -> exp(ACT) -> zt chain would
otherwise stall the PE every key block.  So the projection (Q/K/V) and
output (Wo) matmul chains are broken into 2-matmul units and woven as
independent filler work between the attention blocks: emission order per
block is  sc(j+1) ... fillers ... zt(j),  which keeps the PE queue
saturated with ready work while ACT computes the exp for the block in
flight.  A dummy-matmul warmup burst ramps the PE p-state while the
first DMAs land.

Host: shards/transposes inputs, sums the 4 head-group partial outputs per
batch, adds b_O and the exact b_V fold (softmax rows sum to 1):
  out += b_O + sum_h b_V[h] @ W_O[h].
"""
import numpy as np
import ml_dtypes

import concourse.bacc as bacc
import concourse.mybir as mybir
import concourse.tile as tile
from concourse.bass_utils import run_bass_kernel_spmd

f32 = mybir.dt.float32
bf16 = mybir.dt.bfloat16
fp8 = mybir.dt.float8e4
AF = mybir.ActivationFunctionType

B, S, D, H, DH = 2, 2048, 1024, 16, 64
NCORES = 8
HG = 4                # heads per core
HK = HG * DH          # 256
CH = 512              # query chunk
NCH = S // CH         # 4
KB = 128              # key block
DT = D // 128         # 8

_CACHE = {}


def _build_nc():
    nc = bacc.Bacc(None, target_bir_lowering=False, debug=False,
                   num_devices=NCORES)

    xt_d = nc.dram_tensor("xt", [128, NCH, DT, CH], bf16,
                          kind="ExternalInput")
    wq_d = nc.dram_tensor("wq", [128, 2, DT, 128], bf16, kind="ExternalInput")
    wk_d = nc.dram_tensor("wk", [128, 2, DT, 128], bf16, kind="ExternalInput")
    wv_d = nc.dram_tensor("wv", [128, DT, HK], bf16, kind="ExternalInput")
    wo_d = nc.dram_tensor("wo", [128, 2, D], bf16, kind="ExternalInput")
    bq_d = nc.dram_tensor("bq", [128, 2], f32, kind="ExternalInput")
    bk_d = nc.dram_tensor("bk", [128, 2], f32, kind="ExternalInput")
    mask_d = nc.dram_tensor("mask", [128, 4, 2, CH], fp8,
                            kind="ExternalInput")
    out_d = nc.dram_tensor("outT", [D, S], bf16, kind="ExternalOutput")

    with tile.TileContext(nc) as tc:
        with (
            tc.tile_pool(name="const", bufs=1) as cp,
            tc.tile_pool(name="big", bufs=1) as bp,
            tc.tile_pool(name="work", bufs=3) as wp,
            tc.tile_pool(name="psum", bufs=2, space="PSUM") as pp,
        ):
            # ---- loads
            wq = cp.tile([128, 2, DT, 128], bf16)
            wk = cp.tile([128, 2, DT, 128], bf16)
            wv = cp.tile([128, DT, HK], bf16)
            wo = cp.tile([128, 2, D], bf16)
            bq = cp.tile([128, 2], f32)
            bk = cp.tile([128, 2], f32)
            mask8 = cp.tile([128, 4, 2, CH], fp8)
            mask = cp.tile([128, 4, 2, CH], bf16)
            xt = bp.tile([128, NCH, DT, CH], bf16)

            # Input staging.  Each dma_start is sharded across the ring's
            # 16 engines (per-engine streaming is only a few GB/s), and
            # each issue instruction costs ~0.6us on the issuing queue, so
            # transfers are few and large, split across both rings:
            # SWDGE(gpsimd) carries the critical prefix + chunks 0/1,
            # HWDGE(sync) carries chunks 2/3 and later the outputs.
            # The HWDGE (sync) ring issues fast and ramps to full rate
            # immediately, so it carries the whole critical prefix; the
            # late-ramping SWDGE (gpsimd) ring gets what's needed later.
            nc.sync.dma_start(bq, bq_d[:])
            nc.sync.dma_start(bk, bk_d[:])
            nc.sync.dma_start(wq[:, 0], wq_d[:, 0])
            nc.sync.dma_start(wk[:, 0], wk_d[:, 0])
            # chunk 0 in 4 pieces so the prefix matmuls start on the first
            # di pair instead of waiting for the whole-chunk semaphore
            for p in range(4):
                nc.sync.dma_start(xt[:, 0, 2 * p:2 * p + 2],
                                  xt_d[:, 0, 2 * p:2 * p + 2])
            nc.sync.dma_start(xt[:, 1], xt_d[:, 1])
            nc.sync.dma_start(wv, wv_d[:])
            nc.sync.dma_start(mask8, mask_d[:])
            nc.gpsimd.dma_start(wq[:, 1], wq_d[:, 1])
            nc.gpsimd.dma_start(wk[:, 1], wk_d[:, 1])
            nc.gpsimd.dma_start(wo, wo_d[:])
            nc.gpsimd.dma_start(xt[:, 2], xt_d[:, 2])
            nc.gpsimd.dma_start(xt[:, 3], xt_d[:, 3])

            qt = bp.tile([128, 2, S], bf16)
            kt = bp.tile([128, 2, S], bf16)
            # V padded to 128 columns (cols 65.. zero) so the zT matmul's
            # stationary is 128-wide -> fast weight load
            v = bp.tile([128, S // KB, HG, 128], bf16)
            ztn = bp.tile([128, 2, S], bf16)
            wtile = bp.tile([128, CH], bf16)

            # wtile memset on DVE (simple 2D AP); the strided v-pad
            # memsets go on gpsimd as in the baseline — they run after
            # this queue's DMA issues, well before the first zt needs them
            nc.vector.memset(wtile, 0.0)
            nc.gpsimd.memset(v[:, :, :, DH:DH + 1], 1.0)
            nc.gpsimd.memset(v[:, :, :, DH + 1:], 0.0)
            # mask travels as fp8 (halves bytes on the critical input
            # ring) and is widened once here — the DVE is idle this early
            nc.vector.tensor_copy(mask, mask8)

            # ---- PE p-state warmup: dummy matmuls on zeros while the
            # first input DMAs are still in flight
            for i in range(10):
                ps_w = pp.tile([128, CH], f32, tag="proj", bufs=2,
                               name=f"warm_{i}")
                nc.tensor.matmul(ps_w, wtile[:, 0:128], wtile,
                                 start=True, stop=True)

            # ---------- emission helpers ----------
            def emit_proj(kind, c, m, lo, hi, state):
                """Two matmuls (di=lo..hi-1) of the Q/K projection chain
                for (chunk c, column half m); creates the PSUM tile on the
                first call and appends the bias-cast on the last."""
                w, dst, b = ((wq, qt, bq) if kind == "q" else (wk, kt, bk))
                cs = c * CH
                if lo == 0:
                    state["ps"] = pp.tile([128, CH], f32, tag="proj", bufs=2,
                                          name=f"ps_{kind}_{c}_{m}")
                ps = state["ps"]
                for di in range(lo, hi):
                    nc.tensor.matmul(ps, w[:, m, di, :], xt[:, c, di, :],
                                     start=(di == 0), stop=(di == DT - 1))
                if hi == DT:
                    nc.scalar.activation(dst[:, m, cs:cs + CH], ps,
                                         AF.Identity, bias=b[:, m:m + 1])

            def emit_v(si, lo, hi, state):
                c, sb = si // 4, si % 4
                if lo == 0:
                    state["ps"] = pp.tile([128, HG, DH], f32, tag="proj",
                                          bufs=2, name=f"ps_v_{si}")
                ps = state["ps"]
                for di in range(lo, hi):
                    nc.tensor.matmul(ps, xt[:, c, di, sb * KB:(sb + 1) * KB],
                                     wv[:, di, :],
                                     start=(di == 0), stop=(di == DT - 1))
                if hi == DT:
                    nc.vector.tensor_copy(v[:, si, :, 0:DH], ps)

            def emit_d(c, dt_i, tag="proj", bufs=2, cast_eng=None):
                cs = c * CH
                ps_o = pp.tile([128, CH], f32, tag=tag, bufs=bufs,
                               name=f"ps_o_{c}_{dt_i}")
                for m in range(2):
                    nc.tensor.matmul(ps_o, wo[:, m, dt_i * 128:(dt_i + 1) * 128],
                                     ztn[:, m, cs:cs + CH],
                                     start=(m == 0), stop=(m == 1))
                ost = wp.tile([128, CH], bf16, tag="ost", bufs=4)
                if cast_eng == "act":
                    nc.scalar.activation(ost, ps_o, AF.Identity)
                else:
                    nc.vector.tensor_copy(ost, ps_o)
                nc.sync.dma_start(
                    out_d[dt_i * 128:(dt_i + 1) * 128, cs:cs + CH], ost)

            # ---------- filler unit queue ----------
            # unit = dict(marker, kind, si, thunk); 1 unit ~ 2 matmuls
            units = []

            def add_proj_units(kind, c, m, marker):
                state = {}
                for u in range(4):
                    units.append(dict(
                        marker=marker, kind=kind, si=-1,
                        thunk=(lambda kind=kind, c=c, m=m, u=u, state=state:
                               emit_proj(kind, c, m, 2 * u, 2 * u + 2, state))))

            def add_v_units(si, marker):
                state = {}
                for u in range(4):
                    units.append(dict(
                        marker=marker, kind="v", si=si,
                        thunk=(lambda si=si, u=u, state=state:
                               emit_v(si, 2 * u, 2 * u + 2, state))))

            def add_d_units(c, marker, lo=0, hi=DT):
                for dt_i in range(lo, hi):
                    units.append(dict(
                        marker=marker, kind="d", si=-1,
                        thunk=(lambda c=c, dt_i=dt_i: emit_d(c, dt_i))))

            # marker = section index (sections run c-major, hp-minor)
            add_v_units(0, 0); add_v_units(1, 0)
            add_v_units(2, 0); add_v_units(3, 0)
            add_proj_units("q", 0, 1, 0); add_proj_units("k", 0, 1, 0)
            add_proj_units("q", 1, 0, 1); add_proj_units("k", 1, 0, 1)
            add_v_units(4, 1); add_v_units(5, 1)
            add_v_units(6, 2); add_v_units(7, 2)
            add_proj_units("q", 1, 1, 2); add_proj_units("k", 1, 1, 2)
            add_proj_units("q", 2, 0, 3); add_proj_units("k", 2, 0, 3)
            add_d_units(0, 3)
            add_v_units(8, 4); add_v_units(9, 4)
            add_v_units(10, 4); add_v_units(11, 4)
            add_proj_units("q", 2, 1, 4); add_proj_units("k", 2, 1, 4)
            add_proj_units("q", 3, 0, 5); add_proj_units("k", 3, 0, 5)
            add_v_units(12, 6); add_v_units(13, 6)
            add_v_units(14, 6); add_v_units(15, 6)
            add_proj_units("q", 3, 1, 6); add_proj_units("k", 3, 1, 6)
            # D(c1) weaves into the last section; all of D(c2) is held
            # back (marker 8) and drained right before the final normalize
            # so the PE has ~16 matmuls of work while that chain runs
            add_d_units(1, 7)
            add_d_units(2, 8)

            def drain_until(sec):
                while units and units[0]["marker"] < sec:
                    units.pop(0)["thunk"]()

            def drain_v(sec, j):
                while units and units[0]["marker"] == sec and \
                        units[0]["kind"] == "v" and units[0]["si"] <= j:
                    units.pop(0)["thunk"]()

            # ---------- prefix: minimal critical path to first sc ----------
            ps_q0 = pp.tile([128, CH], f32, tag="proj", bufs=2, name="ps_q00")
            ps_k0 = pp.tile([128, CH], f32, tag="proj", bufs=2, name="ps_k00")
            for di in range(DT):
                nc.tensor.matmul(ps_q0, wq[:, 0, di, :], xt[:, 0, di, :],
                                 start=(di == 0), stop=(di == DT - 1))
                nc.tensor.matmul(ps_k0, wk[:, 0, di, :], xt[:, 0, di, :],
                                 start=(di == 0), stop=(di == DT - 1))
            nc.scalar.activation(qt[:, 0, 0:CH], ps_q0, AF.Identity,
                                 bias=bq[:, 0:1])
            nc.scalar.activation(kt[:, 0, 0:CH], ps_k0, AF.Identity,
                                 bias=bk[:, 0:1])

            # ---------- attention sections with woven fillers ----------
            sections = [(c, hp) for c in range(NCH) for hp in range(2)]
            for sec, (c, hp) in enumerate(sections):
                drain_until(sec)
                cs = c * CH
                nblk = 4 * c + 4
                m = hp
                last = (sec == len(sections) - 1)
                n_mine = sum(1 for u in units if u["marker"] == sec)
                pace = n_mine / nblk
                acc = [0.0]

                def weave():
                    acc[0] += pace
                    while acc[0] >= 1.0 and units and \
                            units[0]["marker"] <= sec:
                        units.pop(0)["thunk"]()
                        acc[0] -= 1.0

                zt0 = pp.tile([128, CH], f32, tag="zt0", bufs=1,
                              name=f"zt0_{c}_{hp}")
                zt1 = pp.tile([128, CH], f32, tag="zt1", bufs=1,
                              name=f"zt1_{c}_{hp}")
                zts = (zt0, zt1)
                exs = [None] * nblk
                qls = [0] * nblk

                def emit_sc(j):
                    t = j - 4 * c
                    ql = 128 * t if t > 0 else 0
                    qls[j] = ql
                    sc = pp.tile([128, 2, CH], f32, tag="sc")
                    for par in range(2):
                        o = par * 64
                        nc.tensor.matmul(
                            sc[:, par, ql:],
                            kt[o:o + 64, m, j * KB:(j + 1) * KB],
                            qt[o:o + 64, m, cs + ql:cs + CH],
                            start=True, stop=True)
                    ex = wp.tile([128, 2, CH], bf16, tag="ex", bufs=6)
                    nc.scalar.activation(ex[:, :, ql:], sc[:, :, ql:],
                                         AF.Exp, scale=0.125)
                    if t >= 0:
                        qm = ql + 128
                        nc.vector.tensor_mul(ex[:, :, ql:qm],
                                             ex[:, :, ql:qm],
                                             mask[:, t, :, ql:qm])
                    exs[j] = ex

                def emit_zt(j):
                    drain_v(sec, j)
                    ql = qls[j]
                    for par in range(2):
                        h = 2 * hp + par
                        nc.tensor.matmul(
                            zts[par][:, ql:], v[:, j, h, :],
                            exs[j][:, par, ql:],
                            start=(j == 0), stop=(j == nblk - 1))
                    exs[j] = None

                # block loop: sc one ahead of zt, fillers woven between
                emit_sc(0)
                for j in range(1, nblk):
                    emit_sc(j)
                    weave()
                    emit_zt(j - 1)
                weave()
                emit_zt(nblk - 1)

                # held-back tail fillers MUST be emitted before the final
                # normalize: semaphore thresholds are captured at emission
                # time, so emitting them later would chain them behind the
                # normalize's ztn writes
                if last:
                    drain_until(9)

                # normalize: ztn[h] = zt[0:64] / zt[64]; bounce zt+denom to
                # SBUF first so the PSUM accumulator frees for the next
                # section (skip the bounce on the final section).  Engine
                # order: both reciprocals (DVE) first, then both gpsimd
                # broadcasts, then both multiplies, so the three engines
                # pipeline instead of ping-ponging.
                zsrcs = []
                for par in range(2):
                    if last:
                        zsrcs.append(zts[par])
                    else:
                        zs = wp.tile([DH + 1, CH], f32, tag="zs", bufs=3,
                                     name=f"zs_{c}_{2 * hp + par}")
                        nc.vector.tensor_copy(zs, zts[par][0:DH + 1, :])
                        zsrcs.append(zs)
                recs = []
                for par in range(2):
                    srow = wp.tile([1, CH], f32, tag="srow", bufs=3,
                                   name=f"srow_{c}_{2 * hp + par}")
                    nc.vector.tensor_copy(srow, zsrcs[par][DH:DH + 1, :])
                    rec = wp.tile([1, CH], f32, tag="rec", bufs=3,
                                  name=f"rec_{c}_{2 * hp + par}")
                    nc.vector.reciprocal_approx_fast(rec, srow)
                    recs.append(rec)
                bcs = []
                for par in range(2):
                    bc = wp.tile([64, CH], f32, tag="bc", bufs=3,
                                 name=f"bc_{c}_{2 * hp + par}")
                    nc.gpsimd.partition_broadcast(bc, recs[par])
                    bcs.append(bc)
                for par in range(2):
                    o = par * 64
                    nc.vector.tensor_mul(ztn[o:o + 64, m, cs:cs + CH],
                                         zsrcs[par][0:DH, :], bcs[par])

            # ---------- tail: last chunk's output projection ----------
            dtags = [("proj", 2), ("sc", 2), ("zt0", 1), ("zt1", 1)]
            for dt_i in range(DT):
                tg, tb = dtags[dt_i % 4]
                emit_d(NCH - 1, dt_i, tag=tg, bufs=tb,
                       cast_eng=("act" if dt_i % 2 == 0 else None))

    nc.compile()
    return nc


def _tile128(a, inner_shape):
    """[N*128, ...] -> [128, N, ...] partition-major layout."""
    n = a.shape[0] // 128
    return np.ascontiguousarray(
        a.reshape((n, 128) + a.shape[1:]).swapaxes(0, 1)).reshape(
            (128, n) + inner_shape)


def _prep_core(x, W_Q, W_K, W_V, W_O, b_Q, b_K, b, g):
    hs = slice(g * HG, (g + 1) * HG)
    bfl = ml_dtypes.bfloat16

    xtp = np.ascontiguousarray(x[b].T)                       # [D, S]
    xt = _tile128(xtp, (S,)).astype(bfl)                     # [128, DT, S]
    # chunk-major so each chunk is one contiguous transfer
    xtc = np.ascontiguousarray(
        xt.reshape(128, DT, NCH, CH).transpose(0, 2, 1, 3))

    def prep_w(w):                                           # [H,D,dh] slice
        wc = np.ascontiguousarray(
            w[hs].transpose(1, 0, 2).reshape(D, HK))         # [D, HK]
        return _tile128(wc, (HK,)).astype(bfl)               # [128, DT, HK]

    def prep_w_m(w):
        # m-major: [128, 2, DT, 128] so each column half is contiguous
        return np.ascontiguousarray(
            prep_w(w).reshape(128, DT, 2, 128).transpose(0, 2, 1, 3))

    wq, wk, wv = prep_w_m(W_Q), prep_w_m(W_K), prep_w(W_V)
    woc = W_O[hs].reshape(HK, D)                             # [HK, D]
    wo = _tile128(woc, (D,)).astype(bfl)                     # [128, 2, D]

    bq = np.ascontiguousarray(
        b_Q[hs].reshape(HK).reshape(2, 128).T).astype(np.float32)
    bk = np.ascontiguousarray(
        b_K[hs].reshape(HK).reshape(2, 128).T).astype(np.float32)

    r = np.arange(128)[:, None, None]
    f = np.arange(CH)[None, None, :]
    t = np.arange(4)[None, :, None]
    m3 = (f >= r + 128 * t)                                  # [128, 4, CH]
    mask = np.repeat(m3[:, :, None, :], 2, axis=2).astype(
        ml_dtypes.float8_e4m3fn)

    return {"xt": xtc, "wq": wq, "wk": wk, "wv": wv, "wo": wo,
            "bq": bq, "bk": bk, "mask": mask}


def kernel(x, W_Q, W_K, W_V, W_O, b_Q, b_K, b_V, b_O, **run_kwargs):
    x = np.asarray(x, dtype=np.float32)
    W_Q = np.asarray(W_Q, dtype=np.float32)
    W_K = np.asarray(W_K, dtype=np.float32)
    W_V = np.asarray(W_V, dtype=np.float32)
    W_O = np.asarray(W_O, dtype=np.float32)
    b_Q = np.asarray(b_Q, dtype=np.float32)
    b_K = np.asarray(b_K, dtype=np.float32)
    b_V = np.asarray(b_V, dtype=np.float32)
    b_O = np.asarray(b_O, dtype=np.float32)

    if "nc" not in _CACHE:
        _CACHE["nc"] = _build_nc()
    nc = _CACHE["nc"]

    in_maps = []
    for i in range(NCORES):
        b, g = i // HG, i % HG
        in_maps.append(_prep_core(x, W_Q, W_K, W_V, W_O, b_Q, b_K, b, g))

    res = run_bass_kernel_spmd(nc, in_maps, core_ids=list(range(NCORES)),
                               **run_kwargs)

    # exact fold of b_V through W_O (softmax rows sum to 1), plus b_O
    bias = (b_O.astype(np.float64)
            + b_V.reshape(H * DH).astype(np.float64)
            @ W_O.reshape(H * DH, D).astype(np.float64)).astype(np.float32)

    out = np.zeros((B, S, D), dtype=np.float32)
    for i in range(NCORES):
        b = i // HG
        out[b] += res.results[i]["outT"].astype(np.float32).T
    out += bias[None, None, :]
    if run_kwargs:
        return out, res
    return out


# revision 37
# speedup vs baseline: 1.0091x; 1.0091x over previous
"""Multi-head causal attention (B=2, S=2048, D=1024, H=16, dh=64) on 8
Trainium2 NeuronCores.

Sharding: core i handles batch b = i//4 and head group g = i%4 (4 heads
each).  Per core everything is computed in a transposed layout:

  QT = Wq_g^T @ x_b^T          [256(hk), 2048(S)]   (bf16)
  KT = Wk_g^T @ x_b^T          [256(hk), 2048(S)]   (bf16)
  V  = x_b @ Wv_g              [2048(S), 4, 65]     (bf16; col 64 = ones)
  per chunk c (512 queries), head-pair hp, key block j (128 keys):
     scT[par] = KT_h[:,j]^T(lhsT) x QT_h[:,c]   -> PSUM [128, 2, 512]
     expT     = exp(scT/8) (* causal mask when j >= 4c)        (bf16)
     zT_h    += V_aug[j]^T(lhsT) x expT[par]    -> PSUM [65, 512]
     ztn      = zT[0:64] * broadcast(1/s)       [256(hk), 2048] (bf16)
  outT = Wo_g^T(lhsT) x ztn                     [1024(d), 2048] (bf16)

Scheduling: the TRN2 PE p-state only reaches full clock under sustained
back-to-back execution, and the sc -> exp(ACT) -> zt chain would
otherwise stall the PE every key block.  So the projection (Q/K/V) and
output (Wo) matmul chains are broken into 2-matmul units and woven as
independent filler work between the attention blocks: emission order per
block is  sc(j+1) ... fillers ... zt(j),  which keeps the PE queue
saturated with ready work while ACT computes the exp for the block in
flight.  A dummy-matmul warmup burst ramps the PE p-state while the
first DMAs land.

Host: shards/transposes inputs, sums the 4 head-group partial outputs per
batch, adds b_O and the exact b_V fold (softmax rows sum to 1):
  out += b_O + sum_h b_V[h] @ W_O[h].
"""
import numpy as np
import ml_dtypes

import concourse.bacc as bacc
import concourse.mybir as mybir
import concourse.tile as tile
from concourse.bass_utils import run_bass_kernel_spmd

f32 = mybir.dt.float32
bf16 = mybir.dt.bfloat16
fp8 = mybir.dt.float8e4
AF = mybir.ActivationFunctionType

B, S, D, H, DH = 2, 2048, 1024, 16, 64
NCORES = 8
HG = 4                # heads per core
HK = HG * DH          # 256
CH = 512              # query chunk
NCH = S // CH         # 4
KB = 128              # key block
DT = D // 128         # 8

_CACHE = {}


def _build_nc():
    nc = bacc.Bacc(None, target_bir_lowering=False, debug=False,
                   num_devices=NCORES)

    # pre packs the whole critical prefix — wq m0-half (1024), wk m0-half
    # (1024), xt chunk 0 (4096) — into one contiguous tensor so it
    # streams as a single deep-pipelined DMA call
    pre_d = nc.dram_tensor("pre", [128, 2 * DT * 128 + DT * CH], bf16,
                           kind="ExternalInput")
    xt_d = nc.dram_tensor("xt", [128, NCH - 1, DT, CH], bf16,
                          kind="ExternalInput")
    wqk1_d = nc.dram_tensor("wqk1", [128, 2, DT, 128], bf16,
                            kind="ExternalInput")
    wv_d = nc.dram_tensor("wv", [128, DT, HK], bf16, kind="ExternalInput")
    wo_d = nc.dram_tensor("wo", [128, 2, D], bf16, kind="ExternalInput")
    bq_d = nc.dram_tensor("bq", [128, 2], f32, kind="ExternalInput")
    bk_d = nc.dram_tensor("bk", [128, 2], f32, kind="ExternalInput")
    mask_d = nc.dram_tensor("mask", [128, 4, 2, CH], bf16,
                            kind="ExternalInput")
    out_d = nc.dram_tensor("outT", [D, S], bf16, kind="ExternalOutput")

    with tile.TileContext(nc) as tc:
        with (
            tc.tile_pool(name="const", bufs=1) as cp,
            tc.tile_pool(name="big", bufs=1) as bp,
            tc.tile_pool(name="work", bufs=3) as wp,
            tc.tile_pool(name="psum", bufs=2, space="PSUM") as pp,
        ):
            # ---- loads
            pre = cp.tile([128, 2 * DT * 128 + DT * CH], bf16)
            wqk1 = cp.tile([128, 2, DT, 128], bf16)
            wv = cp.tile([128, DT, HK], bf16)
            wo = cp.tile([128, 2, D], bf16)
            bq = cp.tile([128, 2], f32)
            bk = cp.tile([128, 2], f32)
            mask = cp.tile([128, 4, 2, CH], bf16)
            xt = bp.tile([128, NCH - 1, DT, CH], bf16)

            # Input staging.  Per-engine DMA is latency-bound on 4KB
            # packets, so transfers must be few and LARGE to pipeline;
            # calls complete in per-engine FIFO order.  The HWDGE (sync)
            # ring issues fast and carries everything needed in the first
            # ~25us; the late-ramping SWDGE (gpsimd) ring gets the rest.
            nc.sync.dma_start(bq, bq_d[:])
            nc.sync.dma_start(bk, bk_d[:])
            nc.sync.dma_start(pre, pre_d[:])
            nc.sync.dma_start(wv, wv_d[:])
            nc.sync.dma_start(mask, mask_d[:])
            nc.sync.dma_start(xt[:, 0], xt_d[:, 0])
            nc.gpsimd.dma_start(wqk1, wqk1_d[:])
            nc.gpsimd.dma_start(wo, wo_d[:])
            nc.gpsimd.dma_start(xt[:, 1], xt_d[:, 1])
            nc.gpsimd.dma_start(xt[:, 2], xt_d[:, 2])

            def w0_ap(kind, di):
                off = (0 if kind == "q" else DT * 128) + di * 128
                return pre[:, off:off + 128]

            def xt_ap(c, di, lo=0, hi=CH):
                if c == 0:
                    base = 2 * DT * 128 + di * CH
                    return pre[:, base + lo:base + hi]
                return xt[:, c - 1, di, lo:hi]

            qt = bp.tile([128, 2, S], bf16)
            kt = bp.tile([128, 2, S], bf16)
            # V padded to 128 columns (cols 65.. zero) so the zT matmul's
            # stationary is 128-wide -> fast weight load
            v = bp.tile([128, S // KB, HG, 128], bf16)
            ztn = bp.tile([128, 2, S], bf16)
            wtile = bp.tile([128, CH], bf16)

            # wtile memset on DVE (simple 2D AP); the strided v-pad
            # memsets go on gpsimd as in the baseline — they run after
            # this queue's DMA issues, well before the first zt needs them
            nc.vector.memset(wtile, 0.0)
            nc.gpsimd.memset(v[:, :, :, DH:DH + 1], 1.0)
            nc.gpsimd.memset(v[:, :, :, DH + 1:], 0.0)

            # ---- PE p-state warmup: dummy matmuls on zeros while the
            # first input DMAs are still in flight
            for i in range(10):
                ps_w = pp.tile([128, CH], f32, tag="proj", bufs=2,
                               name=f"warm_{i}")
                nc.tensor.matmul(ps_w, wtile[:, 0:128], wtile,
                                 start=True, stop=True)

            # ---------- emission helpers ----------
            def emit_proj(kind, c, m, lo, hi, state):
                """Two matmuls (di=lo..hi-1) of the Q/K projection chain
                for (chunk c, column half m); creates the PSUM tile on the
                first call and appends the bias-cast on the last."""
                dst, b = ((qt, bq) if kind == "q" else (kt, bk))
                cs = c * CH
                if lo == 0:
                    state["ps"] = pp.tile([128, CH], f32, tag="proj", bufs=2,
                                          name=f"ps_{kind}_{c}_{m}")
                ps = state["ps"]
                for di in range(lo, hi):
                    w_ap = (w0_ap(kind, di) if m == 0 else
                            wqk1[:, (0 if kind == "q" else 1), di, :])
                    nc.tensor.matmul(ps, w_ap, xt_ap(c, di),
                                     start=(di == 0), stop=(di == DT - 1))
                if hi == DT:
                    nc.scalar.activation(dst[:, m, cs:cs + CH], ps,
                                         AF.Identity, bias=b[:, m:m + 1])

            def emit_v(si, lo, hi, state):
                c, sb = si // 4, si % 4
                if lo == 0:
                    state["ps"] = pp.tile([128, HG, DH], f32, tag="proj",
                                          bufs=2, name=f"ps_v_{si}")
                ps = state["ps"]
                for di in range(lo, hi):
                    nc.tensor.matmul(ps, xt_ap(c, di, sb * KB, (sb + 1) * KB),
                                     wv[:, di, :],
                                     start=(di == 0), stop=(di == DT - 1))
                if hi == DT:
                    nc.vector.tensor_copy(v[:, si, :, 0:DH], ps)

            def emit_d(c, dt_i, tag="proj", bufs=2, cast_eng=None):
                cs = c * CH
                ps_o = pp.tile([128, CH], f32, tag=tag, bufs=bufs,
                               name=f"ps_o_{c}_{dt_i}")
                for m in range(2):
                    nc.tensor.matmul(ps_o, wo[:, m, dt_i * 128:(dt_i + 1) * 128],
                                     ztn[:, m, cs:cs + CH],
                                     start=(m == 0), stop=(m == 1))
                ost = wp.tile([128, CH], bf16, tag="ost", bufs=4)
                if cast_eng == "act":
                    nc.scalar.activation(ost, ps_o, AF.Identity)
                else:
                    nc.vector.tensor_copy(ost, ps_o)
                nc.sync.dma_start(
                    out_d[dt_i * 128:(dt_i + 1) * 128, cs:cs + CH], ost)

            # ---------- filler unit queue ----------
            # unit = dict(marker, kind, si, thunk); 1 unit ~ 2 matmuls
            units = []

            def add_proj_units(kind, c, m, marker):
                state = {}
                for u in range(4):
                    units.append(dict(
                        marker=marker, kind=kind, si=-1,
                        thunk=(lambda kind=kind, c=c, m=m, u=u, state=state:
                               emit_proj(kind, c, m, 2 * u, 2 * u + 2, state))))

            def add_v_units(si, marker):
                state = {}
                for u in range(4):
                    units.append(dict(
                        marker=marker, kind="v", si=si,
                        thunk=(lambda si=si, u=u, state=state:
                               emit_v(si, 2 * u, 2 * u + 2, state))))

            def add_d_units(c, marker, lo=0, hi=DT):
                for dt_i in range(lo, hi):
                    units.append(dict(
                        marker=marker, kind="d", si=-1,
                        thunk=(lambda c=c, dt_i=dt_i: emit_d(c, dt_i))))

            # marker = section index (sections run c-major, hp-minor)
            add_v_units(0, 0); add_v_units(1, 0)
            add_v_units(2, 0); add_v_units(3, 0)
            add_proj_units("q", 0, 1, 0); add_proj_units("k", 0, 1, 0)
            add_proj_units("q", 1, 0, 1); add_proj_units("k", 1, 0, 1)
            add_v_units(4, 1); add_v_units(5, 1)
            add_v_units(6, 2); add_v_units(7, 2)
            add_proj_units("q", 1, 1, 2); add_proj_units("k", 1, 1, 2)
            add_proj_units("q", 2, 0, 3); add_proj_units("k", 2, 0, 3)
            add_d_units(0, 3)
            add_v_units(8, 4); add_v_units(9, 4)
            add_v_units(10, 4); add_v_units(11, 4)
            add_proj_units("q", 2, 1, 4); add_proj_units("k", 2, 1, 4)
            add_proj_units("q", 3, 0, 5); add_proj_units("k", 3, 0, 5)
            add_v_units(12, 6); add_v_units(13, 6)
            add_v_units(14, 6); add_v_units(15, 6)
            add_proj_units("q", 3, 1, 6); add_proj_units("k", 3, 1, 6)
            # D(c1) weaves into the last section; all of D(c2) is held
            # back (marker 8) and drained right before the final normalize
            # so the PE has ~16 matmuls of work while that chain runs
            add_d_units(1, 7)
            add_d_units(2, 8)

            def drain_until(sec):
                while units and units[0]["marker"] < sec:
                    units.pop(0)["thunk"]()

            def drain_v(sec, j):
                while units and units[0]["marker"] == sec and \
                        units[0]["kind"] == "v" and units[0]["si"] <= j:
                    units.pop(0)["thunk"]()

            # ---------- prefix: minimal critical path to first sc ----------
            ps_q0 = pp.tile([128, CH], f32, tag="proj", bufs=2, name="ps_q00")
            ps_k0 = pp.tile([128, CH], f32, tag="proj", bufs=2, name="ps_k00")
            for di in range(DT):
                nc.tensor.matmul(ps_q0, w0_ap("q", di), xt_ap(0, di),
                                 start=(di == 0), stop=(di == DT - 1))
                nc.tensor.matmul(ps_k0, w0_ap("k", di), xt_ap(0, di),
                                 start=(di == 0), stop=(di == DT - 1))
            nc.scalar.activation(qt[:, 0, 0:CH], ps_q0, AF.Identity,
                                 bias=bq[:, 0:1])
            nc.scalar.activation(kt[:, 0, 0:CH], ps_k0, AF.Identity,
                                 bias=bk[:, 0:1])

            # ---------- attention sections with woven fillers ----------
            sections = [(c, hp) for c in range(NCH) for hp in range(2)]
            for sec, (c, hp) in enumerate(sections):
                drain_until(sec)
                cs = c * CH
                nblk = 4 * c + 4
                m = hp
                last = (sec == len(sections) - 1)
                n_mine = sum(1 for u in units if u["marker"] == sec)
                pace = n_mine / nblk
                acc = [0.0]

                def weave():
                    acc[0] += pace
                    while acc[0] >= 1.0 and units and \
                            units[0]["marker"] <= sec:
                        units.pop(0)["thunk"]()
                        acc[0] -= 1.0

                zt0 = pp.tile([128, CH], f32, tag="zt0", bufs=1,
                              name=f"zt0_{c}_{hp}")
                zt1 = pp.tile([128, CH], f32, tag="zt1", bufs=1,
                              name=f"zt1_{c}_{hp}")
                zts = (zt0, zt1)
                exs = [None] * nblk
                qls = [0] * nblk

                def emit_sc(j):
                    t = j - 4 * c
                    ql = 128 * t if t > 0 else 0
                    qls[j] = ql
                    sc = pp.tile([128, 2, CH], f32, tag="sc")
                    for par in range(2):
                        o = par * 64
                        nc.tensor.matmul(
                            sc[:, par, ql:],
                            kt[o:o + 64, m, j * KB:(j + 1) * KB],
                            qt[o:o + 64, m, cs + ql:cs + CH],
                            start=True, stop=True)
                    ex = wp.tile([128, 2, CH], bf16, tag="ex", bufs=6)
                    nc.scalar.activation(ex[:, :, ql:], sc[:, :, ql:],
                                         AF.Exp, scale=0.125)
                    if t >= 0:
                        qm = ql + 128
                        nc.vector.tensor_mul(ex[:, :, ql:qm],
                                             ex[:, :, ql:qm],
                                             mask[:, t, :, ql:qm])
                    exs[j] = ex

                def emit_zt(j):
                    drain_v(sec, j)
                    ql = qls[j]
                    for par in range(2):
                        h = 2 * hp + par
                        nc.tensor.matmul(
                            zts[par][:, ql:], v[:, j, h, :],
                            exs[j][:, par, ql:],
                            start=(j == 0), stop=(j == nblk - 1))
                    exs[j] = None

                # block loop: sc one ahead of zt, fillers woven between
                emit_sc(0)
                for j in range(1, nblk):
                    emit_sc(j)
                    weave()
                    emit_zt(j - 1)
                weave()
                emit_zt(nblk - 1)

                # held-back tail fillers MUST be emitted before the final
                # normalize: semaphore thresholds are captured at emission
                # time, so emitting them later would chain them behind the
                # normalize's ztn writes
                if last:
                    drain_until(9)

                # normalize: ztn[h] = zt[0:64] / zt[64]; bounce zt+denom to
                # SBUF first so the PSUM accumulator frees for the next
                # section (skip the bounce on the final section).  Engine
                # order: both reciprocals (DVE) first, then both gpsimd
                # broadcasts, then both multiplies, so the three engines
                # pipeline instead of ping-ponging.
                zsrcs = []
                for par in range(2):
                    if last:
                        zsrcs.append(zts[par])
                    else:
                        zs = wp.tile([DH + 1, CH], f32, tag="zs", bufs=3,
                                     name=f"zs_{c}_{2 * hp + par}")
                        nc.vector.tensor_copy(zs, zts[par][0:DH + 1, :])
                        zsrcs.append(zs)
                recs = []
                for par in range(2):
                    srow = wp.tile([1, CH], f32, tag="srow", bufs=3,
                                   name=f"srow_{c}_{2 * hp + par}")
                    nc.vector.tensor_copy(srow, zsrcs[par][DH:DH + 1, :])
                    rec = wp.tile([1, CH], f32, tag="rec", bufs=3,
                                  name=f"rec_{c}_{2 * hp + par}")
                    nc.vector.reciprocal_approx_fast(rec, srow)
                    recs.append(rec)
                bcs = []
                for par in range(2):
                    bc = wp.tile([64, CH], f32, tag="bc", bufs=3,
                                 name=f"bc_{c}_{2 * hp + par}")
                    nc.gpsimd.partition_broadcast(bc, recs[par])
                    bcs.append(bc)
                for par in range(2):
                    o = par * 64
                    nc.vector.tensor_mul(ztn[o:o + 64, m, cs:cs + CH],
                                         zsrcs[par][0:DH, :], bcs[par])

            # ---------- tail: last chunk's output projection ----------
            dtags = [("proj", 2), ("sc", 2), ("zt0", 1), ("zt1", 1)]
            for dt_i in range(DT):
                tg, tb = dtags[dt_i % 4]
                emit_d(NCH - 1, dt_i, tag=tg, bufs=tb,
                       cast_eng=("act" if dt_i % 2 == 0 else None))

    nc.compile()
    return nc


def _tile128(a, inner_shape):
    """[N*128, ...] -> [128, N, ...] partition-major layout."""
    n = a.shape[0] // 128
    return np.ascontiguousarray(
        a.reshape((n, 128) + a.shape[1:]).swapaxes(0, 1)).reshape(
            (128, n) + inner_shape)


def _prep_core(x, W_Q, W_K, W_V, W_O, b_Q, b_K, b, g):
    hs = slice(g * HG, (g + 1) * HG)
    bfl = ml_dtypes.bfloat16

    xtp = np.ascontiguousarray(x[b].T)                       # [D, S]
    xt = _tile128(xtp, (S,)).astype(bfl)                     # [128, DT, S]
    # chunk-major so each chunk is one contiguous transfer
    xtc = np.ascontiguousarray(
        xt.reshape(128, DT, NCH, CH).transpose(0, 2, 1, 3))

    def prep_w(w):                                           # [H,D,dh] slice
        wc = np.ascontiguousarray(
            w[hs].transpose(1, 0, 2).reshape(D, HK))         # [D, HK]
        return _tile128(wc, (HK,)).astype(bfl)               # [128, DT, HK]

    def prep_w_m(w):
        # m-major: [128, 2, DT, 128] so each column half is contiguous
        return np.ascontiguousarray(
            prep_w(w).reshape(128, DT, 2, 128).transpose(0, 2, 1, 3))

    wq, wk, wv = prep_w_m(W_Q), prep_w_m(W_K), prep_w(W_V)
    # packed prefix: wq m0-half | wk m0-half | xt chunk 0
    pre = np.ascontiguousarray(np.concatenate(
        [wq[:, 0].reshape(128, DT * 128), wk[:, 0].reshape(128, DT * 128),
         xtc[:, 0].reshape(128, DT * CH)], axis=1))
    wqk1 = np.ascontiguousarray(
        np.stack([wq[:, 1], wk[:, 1]], axis=1))              # [128,2,DT,128]
    woc = W_O[hs].reshape(HK, D)                             # [HK, D]
    wo = _tile128(woc, (D,)).astype(bfl)                     # [128, 2, D]

    bq = np.ascontiguousarray(
        b_Q[hs].reshape(HK).reshape(2, 128).T).astype(np.float32)
    bk = np.ascontiguousarray(
        b_K[hs].reshape(HK).reshape(2, 128).T).astype(np.float32)

    r = np.arange(128)[:, None, None]
    f = np.arange(CH)[None, None, :]
    t = np.arange(4)[None, :, None]
    m3 = (f >= r + 128 * t)                                  # [128, 4, CH]
    mask = np.repeat(m3[:, :, None, :], 2, axis=2).astype(bfl)

    return {"pre": pre, "xt": np.ascontiguousarray(xtc[:, 1:]),
            "wqk1": wqk1, "wv": wv, "wo": wo,
            "bq": bq, "bk": bk, "mask": mask}


def kernel(x, W_Q, W_K, W_V, W_O, b_Q, b_K, b_V, b_O, **run_kwargs):
    x = np.asarray(x, dtype=np.float32)
    W_Q = np.asarray(W_Q, dtype=np.float32)
    W_K = np.asarray(W_K, dtype=np.float32)
    W_V = np.asarray(W_V, dtype=np.float32)
    W_O = np.asarray(W_O, dtype=np.float32)
    b_Q = np.asarray(b_Q, dtype=np.float32)
    b_K = np.asarray(b_K, dtype=np.float32)
    b_V = np.asarray(b_V, dtype=np.float32)
    b_O = np.asarray(b_O, dtype=np.float32)

    if "nc" not in _CACHE:
        _CACHE["nc"] = _build_nc()
    nc = _CACHE["nc"]

    in_maps = []
    for i in range(NCORES):
        b, g = i // HG, i % HG
        in_maps.append(_prep_core(x, W_Q, W_K, W_V, W_O, b_Q, b_K, b, g))

    res = run_bass_kernel_spmd(nc, in_maps, core_ids=list(range(NCORES)),
                               **run_kwargs)

    # exact fold of b_V through W_O (softmax rows sum to 1), plus b_O
    bias = (b_O.astype(np.float64)
            + b_V.reshape(H * DH).astype(np.float64)
            @ W_O.reshape(H * DH, D).astype(np.float64)).astype(np.float32)

    out = np.zeros((B, S, D), dtype=np.float32)
    for i in range(NCORES):
        b = i // HG
        out[b] += res.results[i]["outT"].astype(np.float32).T
    out += bias[None, None, :]
    if run_kwargs:
        return out, res
    return out


# revision 45
# speedup vs baseline: 1.0606x; 1.0510x over previous
"""Multi-head causal attention (B=2, S=2048, D=1024, H=16, dh=64) on 8
Trainium2 NeuronCores.

Sharding: core i handles batch b = i//4 and head group g = i%4 (4 heads
each).  Per core everything is computed in a transposed layout:

  QT = Wq_g^T @ x_b^T          [256(hk), 2048(S)]   (bf16)
  KT = Wk_g^T @ x_b^T          [256(hk), 2048(S)]   (bf16)
  V  = x_b @ Wv_g              [2048(S), 4, 65]     (bf16; col 64 = ones)
  per chunk c (512 queries), head-pair hp, key block j (128 keys):
     scT[par] = KT_h[:,j]^T(lhsT) x QT_h[:,c]   -> PSUM [128, 2, 512]
     expT     = exp(scT/8) (* causal mask when j >= 4c)        (bf16)
     zT_h    += V_aug[j]^T(lhsT) x expT[par]    -> PSUM [65, 512]
     ztn      = zT[0:64] * broadcast(1/s)       [256(hk), 2048] (bf16)
  outT = Wo_g^T(lhsT) x ztn                     [1024(d), 2048] (bf16)

Scheduling: the TRN2 PE p-state only reaches full clock under sustained
back-to-back execution, and the sc -> exp(ACT) -> zt chain would
otherwise stall the PE every key block.  So the projection (Q/K/V) and
output (Wo) matmul chains are broken into 2-matmul units and woven as
independent filler work between the attention blocks: emission order per
block is  sc(j+1) ... fillers ... zt(j),  which keeps the PE queue
saturated with ready work while ACT computes the exp for the block in
flight.  A dummy-matmul warmup burst ramps the PE p-state while the
first DMAs land.

Host: shards/transposes inputs, sums the 4 head-group partial outputs per
batch, adds b_O and the exact b_V fold (softmax rows sum to 1):
  out += b_O + sum_h b_V[h] @ W_O[h].
"""
import numpy as np
import ml_dtypes

import concourse.bacc as bacc
import concourse.mybir as mybir
import concourse.tile as tile
from concourse.bass_utils import run_bass_kernel_spmd

f32 = mybir.dt.float32
bf16 = mybir.dt.bfloat16
fp8 = mybir.dt.float8e4
AF = mybir.ActivationFunctionType

B, S, D, H, DH = 2, 2048, 1024, 16, 64
NCORES = 8
HG = 4                # heads per core
HK = HG * DH          # 256
CH = 512              # query chunk
NCH = S // CH         # 4
KB = 128              # key block
DT = D // 128         # 8

_CACHE = {}


def _build_nc():
    nc = bacc.Bacc(None, target_bir_lowering=False, debug=False,
                   num_devices=NCORES)

    # pre packs the whole critical prefix — wq m0-half (1024), wk m0-half
    # (1024), xt chunk 0 (4096) — into one contiguous tensor so it
    # streams as a single deep-pipelined DMA call
    pre_d = nc.dram_tensor("pre", [128, 2 * DT * 128 + DT * CH], bf16,
                           kind="ExternalInput")
    xt_d = nc.dram_tensor("xt", [128, NCH - 1, DT, CH], bf16,
                          kind="ExternalInput")
    wqk1_d = nc.dram_tensor("wqk1", [128, 2, DT, 128], bf16,
                            kind="ExternalInput")
    wv_d = nc.dram_tensor("wv", [128, DT, HK], bf16, kind="ExternalInput")
    wo_d = nc.dram_tensor("wo", [128, 2, D], bf16, kind="ExternalInput")
    bqk_d = nc.dram_tensor("bqk", [128, 4], f32, kind="ExternalInput")
    mask_d = nc.dram_tensor("mask", [128, 4, 2, CH], bf16,
                            kind="ExternalInput")
    # outputs leave as one big call per chunk: [partition, dt, chunk, q]
    out_d = nc.dram_tensor("outT", [128, DT, NCH, CH], bf16,
                           kind="ExternalOutput")

    with tile.TileContext(nc) as tc:
        with (
            tc.tile_pool(name="const", bufs=1) as cp,
            tc.tile_pool(name="big", bufs=1) as bp,
            tc.tile_pool(name="work", bufs=3) as wp,
            tc.tile_pool(name="psum", bufs=2, space="PSUM") as pp,
        ):
            # ---- loads
            pre = cp.tile([128, 2 * DT * 128 + DT * CH], bf16)
            wqk1 = cp.tile([128, 2, DT, 128], bf16)
            wv = cp.tile([128, DT, HK], bf16)
            wo = cp.tile([128, 2, D], bf16)
            bqk = cp.tile([128, 4], f32)
            mask = cp.tile([128, 4, 2, CH], bf16)
            xt = bp.tile([128, NCH - 1, DT, CH], bf16)

            # Input staging.  Per-engine DMA is latency-bound on small
            # calls (and each call carries ~2us of fixed per-engine
            # latency), so transfers are few and LARGE.  The critical-path
            # `pre` gets the fast-starting HWDGE (sync) ring almost to
            # itself; everything else rides the SWDGE (gpsimd) ring in
            # the order it is first needed.
            nc.sync.dma_start(pre, pre_d[:])
            nc.sync.dma_start(xt[:, 0], xt_d[:, 0])
            nc.gpsimd.dma_start(bqk, bqk_d[:])
            nc.gpsimd.dma_start(wqk1, wqk1_d[:])
            nc.gpsimd.dma_start(wv, wv_d[:])
            nc.gpsimd.dma_start(mask, mask_d[:])
            nc.gpsimd.dma_start(wo, wo_d[:])
            nc.gpsimd.dma_start(xt[:, 1], xt_d[:, 1])
            nc.gpsimd.dma_start(xt[:, 2], xt_d[:, 2])

            def bias_ap(kind, m):
                off = (0 if kind == "q" else 2) + m
                return bqk[:, off:off + 1]

            def w0_ap(kind, di):
                off = (0 if kind == "q" else DT * 128) + di * 128
                return pre[:, off:off + 128]

            def xt_ap(c, di, lo=0, hi=CH):
                if c == 0:
                    base = 2 * DT * 128 + di * CH
                    return pre[:, base + lo:base + hi]
                return xt[:, c - 1, di, lo:hi]

            qt = bp.tile([128, 2, S], bf16)
            kt = bp.tile([128, 2, S], bf16)
            # V padded to 128 columns (cols 65.. zero) so the zT matmul's
            # stationary is 128-wide -> fast weight load
            v = bp.tile([128, S // KB, HG, 128], bf16)
            ztn = bp.tile([128, 2, S], bf16)
            wtile = bp.tile([128, CH], bf16)

            # wtile memset on DVE (simple 2D AP); the strided v-pad
            # memsets go on gpsimd as in the baseline — they run after
            # this queue's DMA issues, well before the first zt needs them
            nc.vector.memset(wtile, 0.0)
            nc.gpsimd.memset(v[:, :, :, DH:DH + 1], 1.0)
            nc.gpsimd.memset(v[:, :, :, DH + 1:], 0.0)

            # ---- PE p-state warmup: dummy matmuls on zeros while the
            # first input DMAs are still in flight
            for i in range(10):
                ps_w = pp.tile([128, CH], f32, tag="proj", bufs=2,
                               name=f"warm_{i}")
                nc.tensor.matmul(ps_w, wtile[:, 0:128], wtile,
                                 start=True, stop=True)

            # ---------- emission helpers ----------
            def emit_proj(kind, c, m, lo, hi, state):
                """Two matmuls (di=lo..hi-1) of the Q/K projection chain
                for (chunk c, column half m); creates the PSUM tile on the
                first call and appends the bias-cast on the last."""
                dst = qt if kind == "q" else kt
                cs = c * CH
                if lo == 0:
                    state["ps"] = pp.tile([128, CH], f32, tag="proj", bufs=2,
                                          name=f"ps_{kind}_{c}_{m}")
                ps = state["ps"]
                for di in range(lo, hi):
                    w_ap = (w0_ap(kind, di) if m == 0 else
                            wqk1[:, (0 if kind == "q" else 1), di, :])
                    nc.tensor.matmul(ps, w_ap, xt_ap(c, di),
                                     start=(di == 0), stop=(di == DT - 1))
                if hi == DT:
                    nc.scalar.activation(dst[:, m, cs:cs + CH], ps,
                                         AF.Identity, bias=bias_ap(kind, m))

            def emit_v(si, lo, hi, state):
                c, sb = si // 4, si % 4
                if lo == 0:
                    state["ps"] = pp.tile([128, HG, DH], f32, tag="proj",
                                          bufs=2, name=f"ps_v_{si}")
                ps = state["ps"]
                for di in range(lo, hi):
                    nc.tensor.matmul(ps, xt_ap(c, di, sb * KB, (sb + 1) * KB),
                                     wv[:, di, :],
                                     start=(di == 0), stop=(di == DT - 1))
                if hi == DT:
                    nc.vector.tensor_copy(v[:, si, :, 0:DH], ps)

            # output tiles accumulate per chunk and leave in ONE big DMA
            # call per chunk (two half-calls for the last chunk, so the
            # first half starts draining while the second computes)
            d_state = {}

            def emit_d(c, dt_i, tag="proj", bufs=2, cast_eng=None):
                cs = c * CH
                ps_o = pp.tile([128, CH], f32, tag=tag, bufs=bufs,
                               name=f"ps_o_{c}_{dt_i}")
                for m in range(2):
                    nc.tensor.matmul(ps_o, wo[:, m, dt_i * 128:(dt_i + 1) * 128],
                                     ztn[:, m, cs:cs + CH],
                                     start=(m == 0), stop=(m == 1))
                st = d_state.setdefault(c, {
                    "buf": wp.tile([128, DT, CH], bf16, tag="ostb", bufs=2,
                                   name=f"ostb_{c}"),
                    "count": 0})
                if cast_eng == "act":
                    # tail only: the Scalar engine is done with exps there
                    nc.scalar.activation(st["buf"][:, dt_i], ps_o, AF.Identity)
                else:
                    nc.vector.tensor_copy(st["buf"][:, dt_i], ps_o)
                st["count"] += 1
                if c == NCH - 1 and st["count"] == DT // 2:
                    nc.sync.dma_start(out_d[:, 0:DT // 2, c, :],
                                      st["buf"][:, 0:DT // 2])
                elif c == NCH - 1 and st["count"] == DT:
                    nc.sync.dma_start(out_d[:, DT // 2:, c, :],
                                      st["buf"][:, DT // 2:])
                elif st["count"] == DT:
                    nc.sync.dma_start(out_d[:, :, c, :], st["buf"])

            # ---------- filler unit queue ----------
            # unit = dict(marker, kind, si, thunk); 1 unit ~ 2 matmuls
            units = []

            def add_proj_units(kind, c, m, marker):
                state = {}
                for u in range(4):
                    units.append(dict(
                        marker=marker, kind=kind, si=-1,
                        thunk=(lambda kind=kind, c=c, m=m, u=u, state=state:
                               emit_proj(kind, c, m, 2 * u, 2 * u + 2, state))))

            def add_v_units(si, marker):
                state = {}
                for u in range(4):
                    units.append(dict(
                        marker=marker, kind="v", si=si,
                        thunk=(lambda si=si, u=u, state=state:
                               emit_v(si, 2 * u, 2 * u + 2, state))))

            def add_d_units(c, marker, lo=0, hi=DT):
                for dt_i in range(lo, hi):
                    units.append(dict(
                        marker=marker, kind="d", si=-1,
                        thunk=(lambda c=c, dt_i=dt_i: emit_d(c, dt_i))))

            # marker = section index (sections run c-major, hp-minor)
            add_v_units(0, 0); add_v_units(1, 0)
            add_v_units(2, 0); add_v_units(3, 0)
            add_proj_units("q", 0, 1, 0); add_proj_units("k", 0, 1, 0)
            add_proj_units("q", 1, 0, 1); add_proj_units("k", 1, 0, 1)
            add_v_units(4, 1); add_v_units(5, 1)
            add_v_units(6, 2); add_v_units(7, 2)
            add_proj_units("q", 1, 1, 2); add_proj_units("k", 1, 1, 2)
            add_proj_units("q", 2, 0, 3); add_proj_units("k", 2, 0, 3)
            add_d_units(0, 3)
            add_v_units(8, 4); add_v_units(9, 4)
            add_v_units(10, 4); add_v_units(11, 4)
            add_proj_units("q", 2, 1, 4); add_proj_units("k", 2, 1, 4)
            add_proj_units("q", 3, 0, 5); add_proj_units("k", 3, 0, 5)
            add_v_units(12, 6); add_v_units(13, 6)
            add_v_units(14, 6); add_v_units(15, 6)
            add_proj_units("q", 3, 1, 6); add_proj_units("k", 3, 1, 6)
            # D(c1) weaves into the last section; all of D(c2) is held
            # back (marker 8) and drained right before the final normalize
            # so the PE has ~16 matmuls of work while that chain runs
            add_d_units(1, 7)
            add_d_units(2, 8)

            def drain_until(sec):
                while units and units[0]["marker"] < sec:
                    units.pop(0)["thunk"]()

            def drain_v(sec, j):
                while units and units[0]["marker"] == sec and \
                        units[0]["kind"] == "v" and units[0]["si"] <= j:
                    units.pop(0)["thunk"]()

            # ---------- prefix: minimal critical path to first sc ----------
            ps_q0 = pp.tile([128, CH], f32, tag="proj", bufs=2, name="ps_q00")
            ps_k0 = pp.tile([128, CH], f32, tag="proj", bufs=2, name="ps_k00")
            for di in range(DT):
                nc.tensor.matmul(ps_q0, w0_ap("q", di), xt_ap(0, di),
                                 start=(di == 0), stop=(di == DT - 1))
                nc.tensor.matmul(ps_k0, w0_ap("k", di), xt_ap(0, di),
                                 start=(di == 0), stop=(di == DT - 1))
            nc.scalar.activation(qt[:, 0, 0:CH], ps_q0, AF.Identity,
                                 bias=bias_ap("q", 0))
            nc.scalar.activation(kt[:, 0, 0:CH], ps_k0, AF.Identity,
                                 bias=bias_ap("k", 0))

            # ---------- attention sections with woven fillers ----------
            sections = [(c, hp) for c in range(NCH) for hp in range(2)]
            for sec, (c, hp) in enumerate(sections):
                drain_until(sec)
                cs = c * CH
                nblk = 4 * c + 4
                m = hp
                last = (sec == len(sections) - 1)
                n_mine = sum(1 for u in units if u["marker"] == sec)
                pace = n_mine / nblk
                acc = [0.0]

                def weave():
                    acc[0] += pace
                    while acc[0] >= 1.0 and units and \
                            units[0]["marker"] <= sec:
                        units.pop(0)["thunk"]()
                        acc[0] -= 1.0

                zt0 = pp.tile([128, CH], f32, tag="zt0", bufs=1,
                              name=f"zt0_{c}_{hp}")
                zt1 = pp.tile([128, CH], f32, tag="zt1", bufs=1,
                              name=f"zt1_{c}_{hp}")
                zts = (zt0, zt1)
                exs = [None] * nblk
                qls = [0] * nblk

                def emit_sc(j):
                    t = j - 4 * c
                    ql = 128 * t if t > 0 else 0
                    qls[j] = ql
                    sc = pp.tile([128, 2, CH], f32, tag="sc")
                    for par in range(2):
                        o = par * 64
                        nc.tensor.matmul(
                            sc[:, par, ql:],
                            kt[o:o + 64, m, j * KB:(j + 1) * KB],
                            qt[o:o + 64, m, cs + ql:cs + CH],
                            start=True, stop=True)
                    ex = wp.tile([128, 2, CH], bf16, tag="ex", bufs=6)
                    nc.scalar.activation(ex[:, :, ql:], sc[:, :, ql:],
                                         AF.Exp, scale=0.125)
                    if t >= 0:
                        qm = ql + 128
                        nc.vector.tensor_mul(ex[:, :, ql:qm],
                                             ex[:, :, ql:qm],
                                             mask[:, t, :, ql:qm])
                    exs[j] = ex

                def emit_zt(j):
                    drain_v(sec, j)
                    ql = qls[j]
                    for par in range(2):
                        h = 2 * hp + par
                        nc.tensor.matmul(
                            zts[par][:, ql:], v[:, j, h, :],
                            exs[j][:, par, ql:],
                            start=(j == 0), stop=(j == nblk - 1))
                    exs[j] = None

                # block loop: sc one ahead of zt, fillers woven between
                emit_sc(0)
                for j in range(1, nblk):
                    emit_sc(j)
                    weave()
                    emit_zt(j - 1)
                weave()
                emit_zt(nblk - 1)

                # held-back tail fillers MUST be emitted before the final
                # normalize: semaphore thresholds are captured at emission
                # time, so emitting them later would chain them behind the
                # normalize's ztn writes
                if last:
                    drain_until(9)

                # normalize: ztn[h] = zt[0:64] / zt[64]; bounce zt+denom to
                # SBUF first so the PSUM accumulator frees for the next
                # section (skip the bounce on the final section).  Engine
                # order: both reciprocals (DVE) first, then both gpsimd
                # broadcasts, then both multiplies, so the three engines
                # pipeline instead of ping-ponging.
                zsrcs = []
                for par in range(2):
                    if last:
                        zsrcs.append(zts[par])
                    else:
                        zs = wp.tile([DH + 1, CH], f32, tag="zs", bufs=3,
                                     name=f"zs_{c}_{2 * hp + par}")
                        nc.vector.tensor_copy(zs, zts[par][0:DH + 1, :])
                        zsrcs.append(zs)
                recs = []
                for par in range(2):
                    srow = wp.tile([1, CH], f32, tag="srow", bufs=3,
                                   name=f"srow_{c}_{2 * hp + par}")
                    nc.vector.tensor_copy(srow, zsrcs[par][DH:DH + 1, :])
                    rec = wp.tile([1, CH], f32, tag="rec", bufs=3,
                                  name=f"rec_{c}_{2 * hp + par}")
                    nc.vector.reciprocal_approx_fast(rec, srow)
                    recs.append(rec)
                bcs = []
                for par in range(2):
                    bc = wp.tile([64, CH], f32, tag="bc", bufs=3,
                                 name=f"bc_{c}_{2 * hp + par}")
                    nc.gpsimd.partition_broadcast(bc, recs[par])
                    bcs.append(bc)
                for par in range(2):
                    o = par * 64
                    nc.vector.tensor_mul(ztn[o:o + 64, m, cs:cs + CH],
                                         zsrcs[par][0:DH, :], bcs[par])

            # ---------- tail: last chunk's output projection ----------
            dtags = [("proj", 2), ("sc", 2), ("zt0", 1), ("zt1", 1)]
            for dt_i in range(DT):
                tg, tb = dtags[dt_i % 4]
                emit_d(NCH - 1, dt_i, tag=tg, bufs=tb,
                       cast_eng=("act" if dt_i % 2 == 0 else None))

    nc.compile()
    return nc


def _tile128(a, inner_shape):
    """[N*128, ...] -> [128, N, ...] partition-major layout."""
    n = a.shape[0] // 128
    return np.ascontiguousarray(
        a.reshape((n, 128) + a.shape[1:]).swapaxes(0, 1)).reshape(
            (128, n) + inner_shape)


def _prep_core(x, W_Q, W_K, W_V, W_O, b_Q, b_K, b, g):
    hs = slice(g * HG, (g + 1) * HG)
    bfl = ml_dtypes.bfloat16

    xtp = np.ascontiguousarray(x[b].T)                       # [D, S]
    xt = _tile128(xtp, (S,)).astype(bfl)                     # [128, DT, S]
    # chunk-major so each chunk is one contiguous transfer
    xtc = np.ascontiguousarray(
        xt.reshape(128, DT, NCH, CH).transpose(0, 2, 1, 3))

    def prep_w(w):                                           # [H,D,dh] slice
        wc = np.ascontiguousarray(
            w[hs].transpose(1, 0, 2).reshape(D, HK))         # [D, HK]
        return _tile128(wc, (HK,)).astype(bfl)               # [128, DT, HK]

    def prep_w_m(w):
        # m-major: [128, 2, DT, 128] so each column half is contiguous
        return np.ascontiguousarray(
            prep_w(w).reshape(128, DT, 2, 128).transpose(0, 2, 1, 3))

    wq, wk, wv = prep_w_m(W_Q), prep_w_m(W_K), prep_w(W_V)
    # packed prefix: wq m0-half | wk m0-half | xt chunk 0
    pre = np.ascontiguousarray(np.concatenate(
        [wq[:, 0].reshape(128, DT * 128), wk[:, 0].reshape(128, DT * 128),
         xtc[:, 0].reshape(128, DT * CH)], axis=1))
    wqk1 = np.ascontiguousarray(
        np.stack([wq[:, 1], wk[:, 1]], axis=1))              # [128,2,DT,128]
    woc = W_O[hs].reshape(HK, D)                             # [HK, D]
    wo = _tile128(woc, (D,)).astype(bfl)                     # [128, 2, D]

    bq = b_Q[hs].reshape(HK).reshape(2, 128).T
    bk = b_K[hs].reshape(HK).reshape(2, 128).T
    bqk = np.ascontiguousarray(
        np.concatenate([bq, bk], axis=1)).astype(np.float32)

    r = np.arange(128)[:, None, None]
    f = np.arange(CH)[None, None, :]
    t = np.arange(4)[None, :, None]
    m3 = (f >= r + 128 * t)                                  # [128, 4, CH]
    mask = np.repeat(m3[:, :, None, :], 2, axis=2).astype(bfl)

    return {"pre": pre, "xt": np.ascontiguousarray(xtc[:, 1:]),
            "wqk1": wqk1, "wv": wv, "wo": wo,
            "bqk": bqk, "mask": mask}


def kernel(x, W_Q, W_K, W_V, W_O, b_Q, b_K, b_V, b_O, **run_kwargs):
    x = np.asarray(x, dtype=np.float32)
    W_Q = np.asarray(W_Q, dtype=np.float32)
    W_K = np.asarray(W_K, dtype=np.float32)
    W_V = np.asarray(W_V, dtype=np.float32)
    W_O = np.asarray(W_O, dtype=np.float32)
    b_Q = np.asarray(b_Q, dtype=np.float32)
    b_K = np.asarray(b_K, dtype=np.float32)
    b_V = np.asarray(b_V, dtype=np.float32)
    b_O = np.asarray(b_O, dtype=np.float32)

    if "nc" not in _CACHE:
        _CACHE["nc"] = _build_nc()
    nc = _CACHE["nc"]

    in_maps = []
    for i in range(NCORES):
        b, g = i // HG, i % HG
        in_maps.append(_prep_core(x, W_Q, W_K, W_V, W_O, b_Q, b_K, b, g))

    res = run_bass_kernel_spmd(nc, in_maps, core_ids=list(range(NCORES)),
                               **run_kwargs)

    # exact fold of b_V through W_O (softmax rows sum to 1), plus b_O
    bias = (b_O.astype(np.float64)
            + b_V.reshape(H * DH).astype(np.float64)
            @ W_O.reshape(H * DH, D).astype(np.float64)).astype(np.float32)

    out = np.zeros((B, S, D), dtype=np.float32)
    for i in range(NCORES):
        b = i // HG
        r = res.results[i]["outT"].astype(np.float32)  # [128, DT, NCH, CH]
        out[b] += r.transpose(2, 3, 1, 0).reshape(S, D)
    out += bias[None, None, :]
    if run_kwargs:
        return out, res
    return out


# revision 47
# speedup vs baseline: 1.0661x; 1.0052x over previous
"""Multi-head causal attention (B=2, S=2048, D=1024, H=16, dh=64) on 8
Trainium2 NeuronCores.

Sharding: core i handles batch b = i//4 and head group g = i%4 (4 heads
each).  Per core everything is computed in a transposed layout:

  QT = Wq_g^T @ x_b^T          [256(hk), 2048(S)]   (bf16)
  KT = Wk_g^T @ x_b^T          [256(hk), 2048(S)]   (bf16)
  V  = x_b @ Wv_g              [2048(S), 4, 65]     (bf16; col 64 = ones)
  per chunk c (512 queries), head-pair hp, key block j (128 keys):
     scT[par] = KT_h[:,j]^T(lhsT) x QT_h[:,c]   -> PSUM [128, 2, 512]
     expT     = exp(scT/8) (* causal mask when j >= 4c)        (bf16)
     zT_h    += V_aug[j]^T(lhsT) x expT[par]    -> PSUM [65, 512]
     ztn      = zT[0:64] * broadcast(1/s)       [256(hk), 2048] (bf16)
  outT = Wo_g^T(lhsT) x ztn                     [1024(d), 2048] (bf16)

Scheduling: the TRN2 PE p-state only reaches full clock under sustained
back-to-back execution, and the sc -> exp(ACT) -> zt chain would
otherwise stall the PE every key block.  So the projection (Q/K/V) and
output (Wo) matmul chains are broken into 2-matmul units and woven as
independent filler work between the attention blocks: emission order per
block is  sc(j+1) ... fillers ... zt(j),  which keeps the PE queue
saturated with ready work while ACT computes the exp for the block in
flight.  A dummy-matmul warmup burst ramps the PE p-state while the
first DMAs land.

Host: shards/transposes inputs, sums the 4 head-group partial outputs per
batch, adds b_O and the exact b_V fold (softmax rows sum to 1):
  out += b_O + sum_h b_V[h] @ W_O[h].
"""
import numpy as np
import ml_dtypes

import concourse.bacc as bacc
import concourse.mybir as mybir
import concourse.tile as tile
from concourse.bass_utils import run_bass_kernel_spmd

f32 = mybir.dt.float32
bf16 = mybir.dt.bfloat16
fp8 = mybir.dt.float8e4
AF = mybir.ActivationFunctionType

B, S, D, H, DH = 2, 2048, 1024, 16, 64
NCORES = 8
HG = 4                # heads per core
HK = HG * DH          # 256
CH = 512              # query chunk
NCH = S // CH         # 4
KB = 128              # key block
DT = D // 128         # 8

_CACHE = {}


def _build_nc():
    nc = bacc.Bacc(None, target_bir_lowering=False, debug=False,
                   num_devices=NCORES)

    # pre packs the whole critical prefix — wq m0-half (1024), wk m0-half
    # (1024), xt chunk 0 (4096) — into one contiguous tensor so it
    # streams as a single deep-pipelined DMA call
    pre_d = nc.dram_tensor("pre", [128, 2 * DT * 128 + DT * CH], bf16,
                           kind="ExternalInput")
    xt_d = nc.dram_tensor("xt", [128, NCH - 1, DT, CH], bf16,
                          kind="ExternalInput")
    wqk1_d = nc.dram_tensor("wqk1", [128, 2, DT, 128], bf16,
                            kind="ExternalInput")
    wv_d = nc.dram_tensor("wv", [128, DT, HK], bf16, kind="ExternalInput")
    wo_d = nc.dram_tensor("wo", [128, 2, D], bf16, kind="ExternalInput")
    bqk_d = nc.dram_tensor("bqk", [128, 4], f32, kind="ExternalInput")
    mask_d = nc.dram_tensor("mask", [128, 4, 2, CH], bf16,
                            kind="ExternalInput")
    # outputs leave as one big call per chunk: [partition, dt, chunk, q]
    out_d = nc.dram_tensor("outT", [128, DT, NCH, CH], bf16,
                           kind="ExternalOutput")

    with tile.TileContext(nc) as tc:
        with (
            tc.tile_pool(name="const", bufs=1) as cp,
            tc.tile_pool(name="big", bufs=1) as bp,
            tc.tile_pool(name="work", bufs=3) as wp,
            tc.tile_pool(name="psum", bufs=2, space="PSUM") as pp,
        ):
            # ---- loads
            pre = cp.tile([128, 2 * DT * 128 + DT * CH], bf16)
            wqk1 = cp.tile([128, 2, DT, 128], bf16)
            wv = cp.tile([128, DT, HK], bf16)
            wo = cp.tile([128, 2, D], bf16)
            bqk = cp.tile([128, 4], f32)
            mask = cp.tile([128, 4, 2, CH], bf16)
            xt = bp.tile([128, NCH - 1, DT, CH], bf16)

            # Input staging.  Per-engine DMA is latency-bound on small
            # calls (and each call carries ~2us of fixed per-engine
            # latency), so transfers are few and LARGE.  The critical-path
            # `pre` gets the fast-starting HWDGE (sync) ring almost to
            # itself; everything else rides the SWDGE (gpsimd) ring in
            # the order it is first needed.
            nc.sync.dma_start(pre, pre_d[:])
            nc.sync.dma_start(xt[:, 0], xt_d[:, 0])
            nc.gpsimd.dma_start(bqk, bqk_d[:])
            nc.gpsimd.dma_start(wqk1, wqk1_d[:])
            nc.gpsimd.dma_start(wv, wv_d[:])
            nc.gpsimd.dma_start(mask, mask_d[:])
            nc.gpsimd.dma_start(wo, wo_d[:])
            nc.gpsimd.dma_start(xt[:, 1], xt_d[:, 1])
            nc.gpsimd.dma_start(xt[:, 2], xt_d[:, 2])

            def bias_ap(kind, m):
                off = (0 if kind == "q" else 2) + m
                return bqk[:, off:off + 1]

            def w0_ap(kind, di):
                off = (0 if kind == "q" else DT * 128) + di * 128
                return pre[:, off:off + 128]

            def xt_ap(c, di, lo=0, hi=CH):
                if c == 0:
                    base = 2 * DT * 128 + di * CH
                    return pre[:, base + lo:base + hi]
                return xt[:, c - 1, di, lo:hi]

            qt = bp.tile([128, 2, S], bf16)
            kt = bp.tile([128, 2, S], bf16)
            # V padded to 128 columns (cols 65.. zero) so the zT matmul's
            # stationary is 128-wide -> fast weight load
            v = bp.tile([128, S // KB, HG, 128], bf16)
            ztn = bp.tile([128, 2, S], bf16)
            wtile = bp.tile([128, CH], bf16)

            # wtile memset on DVE (simple 2D AP); the strided v-pad
            # memsets go on gpsimd as in the baseline — they run after
            # this queue's DMA issues, well before the first zt needs them
            nc.vector.memset(wtile, 0.0)
            nc.gpsimd.memset(v[:, :, :, DH:DH + 1], 1.0)
            nc.gpsimd.memset(v[:, :, :, DH + 1:], 0.0)

            # ---- PE p-state warmup: dummy matmuls on zeros while the
            # first input DMAs are still in flight
            for i in range(10):
                ps_w = pp.tile([128, CH], f32, tag="proj", bufs=2,
                               name=f"warm_{i}")
                nc.tensor.matmul(ps_w, wtile[:, 0:128], wtile,
                                 start=True, stop=True)

            # ---------- emission helpers ----------
            def emit_proj(kind, c, m, lo, hi, state):
                """Two matmuls (di=lo..hi-1) of the Q/K projection chain
                for (chunk c, column half m); creates the PSUM tile on the
                first call and appends the bias-cast on the last."""
                dst = qt if kind == "q" else kt
                cs = c * CH
                if lo == 0:
                    state["ps"] = pp.tile([128, CH], f32, tag="proj", bufs=2,
                                          name=f"ps_{kind}_{c}_{m}")
                ps = state["ps"]
                for di in range(lo, hi):
                    w_ap = (w0_ap(kind, di) if m == 0 else
                            wqk1[:, (0 if kind == "q" else 1), di, :])
                    nc.tensor.matmul(ps, w_ap, xt_ap(c, di),
                                     start=(di == 0), stop=(di == DT - 1))
                if hi == DT:
                    nc.scalar.activation(dst[:, m, cs:cs + CH], ps,
                                         AF.Identity, bias=bias_ap(kind, m))

            def emit_v(si, lo, hi, state):
                c, sb = si // 4, si % 4
                if lo == 0:
                    state["ps"] = pp.tile([128, HG, DH], f32, tag="proj",
                                          bufs=2, name=f"ps_v_{si}")
                ps = state["ps"]
                for di in range(lo, hi):
                    nc.tensor.matmul(ps, xt_ap(c, di, sb * KB, (sb + 1) * KB),
                                     wv[:, di, :],
                                     start=(di == 0), stop=(di == DT - 1))
                if hi == DT:
                    nc.vector.tensor_copy(v[:, si, :, 0:DH], ps)

            # output tiles accumulate per chunk and leave in ONE big DMA
            # call per chunk (two half-calls for the last chunk, so the
            # first half starts draining while the second computes)
            d_state = {}

            def emit_d(c, dt_i, tag="proj", bufs=2, cast_eng=None):
                cs = c * CH
                ps_o = pp.tile([128, CH], f32, tag=tag, bufs=bufs,
                               name=f"ps_o_{c}_{dt_i}")
                for m in range(2):
                    nc.tensor.matmul(ps_o, wo[:, m, dt_i * 128:(dt_i + 1) * 128],
                                     ztn[:, m, cs:cs + CH],
                                     start=(m == 0), stop=(m == 1))
                st = d_state.setdefault(c, {
                    "buf": wp.tile([128, DT, CH], bf16, tag="ostb", bufs=2,
                                   name=f"ostb_{c}"),
                    "count": 0})
                if cast_eng == "act":
                    # tail only: the Scalar engine is done with exps there
                    nc.scalar.activation(st["buf"][:, dt_i], ps_o, AF.Identity)
                else:
                    nc.vector.tensor_copy(st["buf"][:, dt_i], ps_o)
                st["count"] += 1
                if c == NCH - 1:
                    # last chunk drains in 2-dt pieces so the final call
                    # is small and the ring finishes right behind the PE
                    if st["count"] % 2 == 0:
                        lo = st["count"] - 2
                        nc.sync.dma_start(out_d[:, lo:lo + 2, c, :],
                                          st["buf"][:, lo:lo + 2])
                elif st["count"] == DT:
                    nc.sync.dma_start(out_d[:, :, c, :], st["buf"])

            # ---------- filler unit queue ----------
            # unit = dict(marker, kind, si, thunk); 1 unit ~ 2 matmuls
            units = []

            def add_proj_units(kind, c, m, marker):
                state = {}
                for u in range(4):
                    units.append(dict(
                        marker=marker, kind=kind, si=-1,
                        thunk=(lambda kind=kind, c=c, m=m, u=u, state=state:
                               emit_proj(kind, c, m, 2 * u, 2 * u + 2, state))))

            def add_v_units(si, marker):
                state = {}
                for u in range(4):
                    units.append(dict(
                        marker=marker, kind="v", si=si,
                        thunk=(lambda si=si, u=u, state=state:
                               emit_v(si, 2 * u, 2 * u + 2, state))))

            def add_d_units(c, marker, lo=0, hi=DT):
                for dt_i in range(lo, hi):
                    units.append(dict(
                        marker=marker, kind="d", si=-1,
                        thunk=(lambda c=c, dt_i=dt_i: emit_d(c, dt_i))))

            # marker = section index (sections run c-major, hp-minor)
            add_v_units(0, 0); add_v_units(1, 0)
            add_v_units(2, 0); add_v_units(3, 0)
            add_proj_units("q", 0, 1, 0); add_proj_units("k", 0, 1, 0)
            add_proj_units("q", 1, 0, 1); add_proj_units("k", 1, 0, 1)
            add_v_units(4, 1); add_v_units(5, 1)
            add_v_units(6, 2); add_v_units(7, 2)
            add_proj_units("q", 1, 1, 2); add_proj_units("k", 1, 1, 2)
            add_proj_units("q", 2, 0, 3); add_proj_units("k", 2, 0, 3)
            add_d_units(0, 3)
            add_v_units(8, 4); add_v_units(9, 4)
            add_v_units(10, 4); add_v_units(11, 4)
            add_proj_units("q", 2, 1, 4); add_proj_units("k", 2, 1, 4)
            add_proj_units("q", 3, 0, 5); add_proj_units("k", 3, 0, 5)
            add_v_units(12, 6); add_v_units(13, 6)
            add_v_units(14, 6); add_v_units(15, 6)
            add_proj_units("q", 3, 1, 6); add_proj_units("k", 3, 1, 6)
            # D(c1) weaves into the last section; all of D(c2) is held
            # back (marker 8) and drained right before the final normalize
            # so the PE has ~16 matmuls of work while that chain runs
            add_d_units(1, 7)
            add_d_units(2, 8)

            def drain_until(sec):
                while units and units[0]["marker"] < sec:
                    units.pop(0)["thunk"]()

            def drain_v(sec, j):
                while units and units[0]["marker"] == sec and \
                        units[0]["kind"] == "v" and units[0]["si"] <= j:
                    units.pop(0)["thunk"]()

            # ---------- prefix: minimal critical path to first sc ----------
            ps_q0 = pp.tile([128, CH], f32, tag="proj", bufs=2, name="ps_q00")
            ps_k0 = pp.tile([128, CH], f32, tag="proj", bufs=2, name="ps_k00")
            for di in range(DT):
                nc.tensor.matmul(ps_q0, w0_ap("q", di), xt_ap(0, di),
                                 start=(di == 0), stop=(di == DT - 1))
                nc.tensor.matmul(ps_k0, w0_ap("k", di), xt_ap(0, di),
                                 start=(di == 0), stop=(di == DT - 1))
            nc.scalar.activation(qt[:, 0, 0:CH], ps_q0, AF.Identity,
                                 bias=bias_ap("q", 0))
            nc.scalar.activation(kt[:, 0, 0:CH], ps_k0, AF.Identity,
                                 bias=bias_ap("k", 0))

            # ---------- attention sections with woven fillers ----------
            sections = [(c, hp) for c in range(NCH) for hp in range(2)]
            for sec, (c, hp) in enumerate(sections):
                drain_until(sec)
                cs = c * CH
                nblk = 4 * c + 4
                m = hp
                last = (sec == len(sections) - 1)
                n_mine = sum(1 for u in units if u["marker"] == sec)
                pace = n_mine / nblk
                # back-load the final section's fillers: its tail blocks
                # have no other PE work, and a clock droop right before
                # the output projection is costlier than one early on
                acc = [-0.5 * n_mine if last else 0.0]

                def weave():
                    acc[0] += pace
                    while acc[0] >= 1.0 and units and \
                            units[0]["marker"] <= sec:
                        units.pop(0)["thunk"]()
                        acc[0] -= 1.0

                zt0 = pp.tile([128, CH], f32, tag="zt0", bufs=1,
                              name=f"zt0_{c}_{hp}")
                zt1 = pp.tile([128, CH], f32, tag="zt1", bufs=1,
                              name=f"zt1_{c}_{hp}")
                zts = (zt0, zt1)
                exs = [None] * nblk
                qls = [0] * nblk

                def emit_sc(j):
                    t = j - 4 * c
                    ql = 128 * t if t > 0 else 0
                    qls[j] = ql
                    sc = pp.tile([128, 2, CH], f32, tag="sc")
                    for par in range(2):
                        o = par * 64
                        nc.tensor.matmul(
                            sc[:, par, ql:],
                            kt[o:o + 64, m, j * KB:(j + 1) * KB],
                            qt[o:o + 64, m, cs + ql:cs + CH],
                            start=True, stop=True)
                    ex = wp.tile([128, 2, CH], bf16, tag="ex", bufs=6)
                    nc.scalar.activation(ex[:, :, ql:], sc[:, :, ql:],
                                         AF.Exp, scale=0.125)
                    if t >= 0:
                        qm = ql + 128
                        nc.vector.tensor_mul(ex[:, :, ql:qm],
                                             ex[:, :, ql:qm],
                                             mask[:, t, :, ql:qm])
                    exs[j] = ex

                def emit_zt(j):
                    drain_v(sec, j)
                    ql = qls[j]
                    for par in range(2):
                        h = 2 * hp + par
                        nc.tensor.matmul(
                            zts[par][:, ql:], v[:, j, h, :],
                            exs[j][:, par, ql:],
                            start=(j == 0), stop=(j == nblk - 1))
                    exs[j] = None

                # block loop: sc one ahead of zt, fillers woven between
                emit_sc(0)
                for j in range(1, nblk):
                    emit_sc(j)
                    weave()
                    emit_zt(j - 1)
                weave()
                emit_zt(nblk - 1)

                # held-back tail fillers MUST be emitted before the final
                # normalize: semaphore thresholds are captured at emission
                # time, so emitting them later would chain them behind the
                # normalize's ztn writes
                if last:
                    drain_until(9)

                # normalize: ztn[h] = zt[0:64] / zt[64]; bounce zt+denom to
                # SBUF first so the PSUM accumulator frees for the next
                # section (skip the bounce on the final section).  Engine
                # order: both reciprocals (DVE) first, then both gpsimd
                # broadcasts, then both multiplies, so the three engines
                # pipeline instead of ping-ponging.
                zsrcs = []
                for par in range(2):
                    if last:
                        zsrcs.append(zts[par])
                    else:
                        zs = wp.tile([DH + 1, CH], f32, tag="zs", bufs=3,
                                     name=f"zs_{c}_{2 * hp + par}")
                        nc.vector.tensor_copy(zs, zts[par][0:DH + 1, :])
                        zsrcs.append(zs)
                recs = []
                for par in range(2):
                    srow = wp.tile([1, CH], f32, tag="srow", bufs=3,
                                   name=f"srow_{c}_{2 * hp + par}")
                    nc.vector.tensor_copy(srow, zsrcs[par][DH:DH + 1, :])
                    rec = wp.tile([1, CH], f32, tag="rec", bufs=3,
                                  name=f"rec_{c}_{2 * hp + par}")
                    nc.vector.reciprocal_approx_fast(rec, srow)
                    recs.append(rec)
                bcs = []
                for par in range(2):
                    bc = wp.tile([64, CH], f32, tag="bc", bufs=3,
                                 name=f"bc_{c}_{2 * hp + par}")
                    nc.gpsimd.partition_broadcast(bc, recs[par])
                    bcs.append(bc)
                for par in range(2):
                    o = par * 64
                    nc.vector.tensor_mul(ztn[o:o + 64, m, cs:cs + CH],
                                         zsrcs[par][0:DH, :], bcs[par])

            # ---------- tail: last chunk's output projection ----------
            dtags = [("proj", 2), ("sc", 2), ("zt0", 1), ("zt1", 1)]
            for dt_i in range(DT):
                tg, tb = dtags[dt_i % 4]
                emit_d(NCH - 1, dt_i, tag=tg, bufs=tb,
                       cast_eng=("act" if dt_i % 2 == 0 else None))

    nc.compile()
    return nc


def _tile128(a, inner_shape):
    """[N*128, ...] -> [128, N, ...] partition-major layout."""
    n = a.shape[0] // 128
    return np.ascontiguousarray(
        a.reshape((n, 128) + a.shape[1:]).swapaxes(0, 1)).reshape(
            (128, n) + inner_shape)


def _prep_core(x, W_Q, W_K, W_V, W_O, b_Q, b_K, b, g):
    hs = slice(g * HG, (g + 1) * HG)
    bfl = ml_dtypes.bfloat16

    xtp = np.ascontiguousarray(x[b].T)                       # [D, S]
    xt = _tile128(xtp, (S,)).astype(bfl)                     # [128, DT, S]
    # chunk-major so each chunk is one contiguous transfer
    xtc = np.ascontiguousarray(
        xt.reshape(128, DT, NCH, CH).transpose(0, 2, 1, 3))

    def prep_w(w):                                           # [H,D,dh] slice
        wc = np.ascontiguousarray(
            w[hs].transpose(1, 0, 2).reshape(D, HK))         # [D, HK]
        return _tile128(wc, (HK,)).astype(bfl)               # [128, DT, HK]

    def prep_w_m(w):
        # m-major: [128, 2, DT, 128] so each column half is contiguous
        return np.ascontiguousarray(
            prep_w(w).reshape(128, DT, 2, 128).transpose(0, 2, 1, 3))

    wq, wk, wv = prep_w_m(W_Q), prep_w_m(W_K), prep_w(W_V)
    # packed prefix: wq m0-half | wk m0-half | xt chunk 0
    pre = np.ascontiguousarray(np.concatenate(
        [wq[:, 0].reshape(128, DT * 128), wk[:, 0].reshape(128, DT * 128),
         xtc[:, 0].reshape(128, DT * CH)], axis=1))
    wqk1 = np.ascontiguousarray(
        np.stack([wq[:, 1], wk[:, 1]], axis=1))              # [128,2,DT,128]
    woc = W_O[hs].reshape(HK, D)                             # [HK, D]
    wo = _tile128(woc, (D,)).astype(bfl)                     # [128, 2, D]

    bq = b_Q[hs].reshape(HK).reshape(2, 128).T
    bk = b_K[hs].reshape(HK).reshape(2, 128).T
    bqk = np.ascontiguousarray(
        np.concatenate([bq, bk], axis=1)).astype(np.float32)

    r = np.arange(128)[:, None, None]
    f = np.arange(CH)[None, None, :]
    t = np.arange(4)[None, :, None]
    m3 = (f >= r + 128 * t)                                  # [128, 4, CH]
    mask = np.repeat(m3[:, :, None, :], 2, axis=2).astype(bfl)

    return {"pre": pre, "xt": np.ascontiguousarray(xtc[:, 1:]),
            "wqk1": wqk1, "wv": wv, "wo": wo,
            "bqk": bqk, "mask": mask}


def kernel(x, W_Q, W_K, W_V, W_O, b_Q, b_K, b_V, b_O, **run_kwargs):
    x = np.asarray(x, dtype=np.float32)
    W_Q = np.asarray(W_Q, dtype=np.float32)
    W_K = np.asarray(W_K, dtype=np.float32)
    W_V = np.asarray(W_V, dtype=np.float32)
    W_O = np.asarray(W_O, dtype=np.float32)
    b_Q = np.asarray(b_Q, dtype=np.float32)
    b_K = np.asarray(b_K, dtype=np.float32)
    b_V = np.asarray(b_V, dtype=np.float32)
    b_O = np.asarray(b_O, dtype=np.float32)

    if "nc" not in _CACHE:
        _CACHE["nc"] = _build_nc()
    nc = _CACHE["nc"]

    in_maps = []
    for i in range(NCORES):
        b, g = i // HG, i % HG
        in_maps.append(_prep_core(x, W_Q, W_K, W_V, W_O, b_Q, b_K, b, g))

    res = run_bass_kernel_spmd(nc, in_maps, core_ids=list(range(NCORES)),
                               **run_kwargs)

    # exact fold of b_V through W_O (softmax rows sum to 1), plus b_O
    bias = (b_O.astype(np.float64)
            + b_V.reshape(H * DH).astype(np.float64)
            @ W_O.reshape(H * DH, D).astype(np.float64)).astype(np.float32)

    out = np.zeros((B, S, D), dtype=np.float32)
    for i in range(NCORES):
        b = i // HG
        r = res.results[i]["outT"].astype(np.float32)  # [128, DT, NCH, CH]
        out[b] += r.transpose(2, 3, 1, 0).reshape(S, D)
    out += bias[None, None, :]
    if run_kwargs:
        return out, res
    return out
